# revision 1
# baseline (speedup 1.0000x reference)
"""HGCN (hyperbolic GCN) 2-layer forward for Trainium2, 8 NeuronCores.

Strategy (graph-parallel, dense-spmm):
  - Nodes padded 10000 -> 10240 and sharded 1280/core (8 cores).
  - segment_sum over the edge list is recast as a dense matmul
    agg = A @ xt with A[dst, src] = sum of edge weights; A is built on the
    host from edge_index/edge_weight and each core streams its
    [10240 x 1280] column slice of A^T (k-major tiles) from HBM.
  - Per layer: HypLinear + logmap0 run on the core's own 1280 nodes,
    the [1280, 256] tangent features are AllGathered (DRAM bounce),
    the spmm accumulates 10 PSUM tiles over 80 k-tiles, and HypAct
    (expmap0/proj/relu-logmap/expmap0/proj) finishes in place.
  - All per-node scalar chains (norms, artanh, tanh, mobius coeffs) are
    batched as [128, 10] column arrays to amortize instruction overhead.

kernel(**inputs) takes the FULL unsharded inputs and returns [2, N, D].
"""

import sys

import numpy as np

for _p in ("/opt/trn_rl_repo",):
    if _p not in sys.path:
        sys.path.append(_p)

import concourse.bass as bass  # noqa: E402
import concourse.tile as tile  # noqa: E402
from concourse import bacc, mybir  # noqa: E402
from concourse.bass_utils import run_bass_kernel_spmd  # noqa: E402
from concourse.masks import make_identity  # noqa: E402

AF = mybir.ActivationFunctionType
ALU = mybir.AluOpType
F32 = mybir.dt.float32

NCORES = 8
N = 10000
D = 256
NP = 10240
PC = NP // NCORES      # 1280 nodes per core
NT = PC // 128         # 10 node tiles per core
KT = NP // 128         # 80 contraction tiles
MAXN = 1.0 - 4e-3      # PROJ_EPS clip for c=1
MINN = 1e-15
MM_DT = "bfloat16"     # dtype of the spmm operands ("float32" | "bfloat16")
DEBUG = False          # add intermediate dumps for layer 0


def _mm_np_dtype():
    if MM_DT == "bfloat16":
        import ml_dtypes

        return np.dtype(ml_dtypes.bfloat16)
    return np.dtype(np.float32)


def _mm_bir_dtype():
    return mybir.dt.bfloat16 if MM_DT == "bfloat16" else F32


def build_nc(y2s):
    """Build the per-core Bass program. y2s = (||hyp_b1||^2, ||hyp_b2||^2)."""
    mmdt = _mm_bir_dtype()
    nc = bacc.Bacc("TRN2", target_bir_lowering=False, debug=False,
                   num_devices=NCORES)

    xc = nc.dram_tensor("xc", [NT, 128, D], F32, kind="ExternalInput")
    a_d = nc.dram_tensor("a", [KT, 128, PC], mmdt, kind="ExternalInput")
    w1t = nc.dram_tensor("w1t", [2, 128, D], F32, kind="ExternalInput")
    w2t = nc.dram_tensor("w2t", [2, 128, D], F32, kind="ExternalInput")
    hb1 = nc.dram_tensor("hb1", [128, D], F32, kind="ExternalInput")
    hb2 = nc.dram_tensor("hb2", [128, D], F32, kind="ExternalInput")
    e1_d = nc.dram_tensor("e1", [NT, 128, D], F32, kind="ExternalOutput")
    e2_d = nc.dram_tensor("e2", [NT, 128, D], F32, kind="ExternalOutput")
    dbg = {}
    if DEBUG:
        for nm, shp in [("dbg_h", [NT, 128, D]), ("dbg_mx", [NT, 128, D]),
                        ("dbg_xt", [NT, 128, D]), ("dbg_agg", [NT, 128, D]),
                        ("dbg_xtf", [KT, 128, D])]:
            dbg[nm] = nc.dram_tensor(nm, shp, F32, kind="ExternalOutput")

    with tile.TileContext(nc) as tc:
        with (
            tc.tile_pool(name="const", bufs=1) as const,
            tc.tile_pool(name="persist", bufs=1) as persist,
            tc.tile_pool(name="sqp", bufs=3) as sqp,
            tc.tile_pool(name="htp", bufs=4) as htp,
            tc.tile_pool(name="atp", bufs=8) as atp,
            tc.tile_pool(name="pst", bufs=2, space="PSUM") as pst,
            tc.tile_pool(name="psmx", bufs=1, space="PSUM") as psmx,
            tc.tile_pool(name="psagg", bufs=1, space="PSUM") as psagg,
            tc.tile_pool(name="dram", bufs=1, space="DRAM") as dram,
        ):
            ident = const.tile([128, 128], F32, name="ident")
            make_identity(nc, ident)

            w_sb = []
            for li, wd in enumerate((w1t, w2t)):
                w = const.tile([128, 2, D], F32, name=f"w{li}")
                nc.sync.dma_start(w[:], wd.ap().rearrange("k p n -> p k n"))
                w_sb.append(w)
            hb_sb = []
            for li, hd in enumerate((hb1, hb2)):
                h = const.tile([128, D], F32, name=f"hb{li}")
                nc.sync.dma_start(h[:], hd.ap())
                hb_sb.append(h)

            def sc(name):
                return persist.tile([128, NT], F32, name=name)

            def square_accum(src_ap, accum_ap, name):
                s = sqp.tile([128, D], F32, name="sqt", tag="sqt")
                nc.scalar.activation(s[:], src_ap, AF.Square, accum_out=accum_ap)

            def clamp_recip(dst, src, name):
                c = sc(name + "_c")
                nc.vector.tensor_scalar_max(c[:], src[:], MINN)
                nc.vector.reciprocal(dst[:], c[:])

            def artanh_ln(dst, x, name):
                """dst = ln((1+x)/(1-x)); caller owns the 0.5 factor."""
                ap1 = sc(name + "_ap")
                am1 = sc(name + "_am")
                ram = sc(name + "_ram")
                q = sc(name + "_q")
                nc.scalar.activation(ap1[:], x[:], AF.Identity, bias=1.0)
                nc.scalar.activation(am1[:], x[:], AF.Identity, bias=1.0, scale=-1.0)
                nc.vector.reciprocal(ram[:], am1[:])
                nc.vector.tensor_tensor(q[:], ap1[:], ram[:], ALU.mult)
                nc.scalar.activation(dst[:], q[:], AF.Ln)

            # ---------------- encode: h = proj(expmap0(x)) ----------------
            x_sb = persist.tile([128, NT, D], F32, name="x_sb", tag="bigA")
            nc.sync.dma_start(x_sb[:], xc.ap().rearrange("t p d -> p t d"))
            h_all = persist.tile([128, NT, D], F32, name="h_all", tag="bigB")
            xn2 = sc("xn2")
            for t in range(NT):
                square_accum(x_sb[:, t, :], xn2[:, t : t + 1], f"enc{t}")
            un = sc("un")
            nc.scalar.activation(un[:], xn2[:], AF.Sqrt)
            run_ = sc("run")
            clamp_recip(run_, un, "enc_r")
            thx = sc("thx")
            nc.scalar.activation(thx[:], un[:], AF.Tanh)
            mn0 = sc("mn0")
            nc.vector.tensor_scalar_min(mn0[:], thx[:], MAXN)
            s0 = sc("s0")
            nc.vector.tensor_tensor(s0[:], mn0[:], run_[:], ALU.mult)
            for t in range(NT):
                nc.vector.tensor_scalar_mul(h_all[:, t, :], x_sb[:, t, :],
                                            s0[:, t : t + 1])

            def layer(li, h_in, hnorm, e_out_d):
                """One HGCN layer; h_in [128,NT,D] on-ball, hnorm [128,NT] its
                row norms. Returns (e_all, out_norms)."""
                L = f"l{li}_"
                w = w_sb[li]
                hb = hb_sb[li]
                y2 = float(y2s[li])

                # ---- HypLinear matmuls + |mx|^2 ----
                mx_all = persist.tile([128, NT, D], F32, name=L + "mx", tag="bigA")
                mn2 = sc(L + "mn2")
                for t in range(NT):
                    hT = htp.tile([128, 2, 128], F32, name="hT", tag="hT")
                    for kc in range(2):
                        psT = pst.tile([128, 128], F32, name="psT", tag="psT")
                        nc.tensor.transpose(
                            psT[:], h_in[:, t, kc * 128 : (kc + 1) * 128], ident[:])
                        nc.vector.tensor_copy(hT[:, kc, :], psT[:])
                    pmx = psmx.tile([128, D], F32, name="pmx", tag="pmx")
                    nc.tensor.matmul(pmx[:], hT[:, 0, :], w[:, 0, :],
                                     start=True, stop=False)
                    nc.tensor.matmul(pmx[:], hT[:, 1, :], w[:, 1, :],
                                     start=False, stop=True)
                    square_accum(pmx[:], mn2[:, t : t + 1], L + f"mx{t}")
                    nc.vector.tensor_copy(mx_all[:, t, :], pmx[:])

                # ---- SB1: mobius_matvec scalars ----
                mxn = sc(L + "mxn")
                nc.scalar.activation(mxn[:], mn2[:], AF.Sqrt)
                nc.vector.tensor_scalar_max(mxn[:], mxn[:], MINN)
                rxn = sc(L + "rxn")
                clamp_recip(rxn, hnorm, L + "rxn")
                rmxn = sc(L + "rmxn")
                nc.vector.reciprocal(rmxn[:], mxn[:])
                atx = sc(L + "atx")
                artanh_ln(atx, hnorm, L + "atx")
                targ = sc(L + "targ")
                nc.vector.tensor_tensor(targ[:], mxn[:], rxn[:], ALU.mult)
                nc.vector.tensor_tensor(targ[:], targ[:], atx[:], ALU.mult)
                th = sc(L + "th")
                nc.scalar.activation(th[:], targ[:], AF.Tanh, scale=0.5)
                sres = sc(L + "sres")
                nc.vector.tensor_tensor(sres[:], th[:], rmxn[:], ALU.mult)
                # proj of res: norm is th (analytically); f1 = min(MAXN/th, 1)
                rth = sc(L + "rth")
                clamp_recip(rth, th, L + "rth")
                f1 = sc(L + "f1")
                nc.vector.tensor_scalar(f1[:], rth[:], MAXN, 1.0, ALU.mult, ALU.min)
                nres = sc(L + "nres")
                nc.vector.tensor_scalar_min(nres[:], th[:], MAXN)
                x2 = sc(L + "x2")
                nc.vector.tensor_tensor(x2[:], nres[:], nres[:], ALU.mult)

                # ---- per tile: xy accumulation (on unscaled mx) ----
                ryp = sc(L + "ryp")
                for t in range(NT):
                    prod = sqp.tile([128, D], F32, name="prodt", tag="prodt")
                    nc.vector.tensor_tensor(prod[:], mx_all[:, t, :], hb[:],
                                            ALU.mult)
                    nc.vector.reduce_sum(ryp[:, t : t + 1], prod[:],
                                         axis=mybir.AxisListType.X)

                # ---- SB2: mobius_add coefficients ----
                xy = sc(L + "xy")
                nc.vector.tensor_tensor(xy[:], ryp[:], sres[:], ALU.mult)
                nc.vector.tensor_tensor(xy[:], xy[:], f1[:], ALU.mult)
                apre = sc(L + "apre")
                nc.vector.tensor_scalar(apre[:], xy[:], 2.0, 1.0 + y2,
                                        ALU.mult, ALU.add)
                alpha = sc(L + "alpha")
                nc.vector.tensor_tensor(alpha[:], apre[:], f1[:], ALU.mult)
                beta = sc(L + "beta")
                nc.scalar.activation(beta[:], x2[:], AF.Identity,
                                     bias=1.0, scale=-1.0)
                den = sc(L + "den")
                nc.vector.tensor_scalar(den[:], x2[:], y2, 1.0, ALU.mult, ALU.add)
                xy2 = sc(L + "xy2")
                nc.vector.tensor_scalar_mul(xy2[:], xy[:], 2.0)
                nc.vector.tensor_tensor(den[:], den[:], xy2[:], ALU.add)
                dinv = sc(L + "dinv")
                clamp_recip(dinv, den, L + "dinv")
                asc = sc(L + "asc")
                nc.vector.tensor_tensor(asc[:], alpha[:], dinv[:], ALU.mult)
                nc.vector.tensor_tensor(asc[:], asc[:], sres[:], ALU.mult)
                bsc = sc(L + "bsc")
                nc.vector.tensor_tensor(bsc[:], beta[:], dinv[:], ALU.mult)

                # ---- per tile: h2 = asc*mx + bsc*hb ; |h2|^2 ----
                h2_all = persist.tile([128, NT, D], F32, name=L + "h2", tag="bigB")
                hn2 = sc(L + "hn2")
                for t in range(NT):
                    t1 = sqp.tile([128, D], F32, name="t1t", tag="t1t")
                    nc.vector.tensor_scalar_mul(t1[:], mx_all[:, t, :],
                                                asc[:, t : t + 1])
                    t2 = sqp.tile([128, D], F32, name="t2t", tag="t2t")
                    nc.scalar.activation(t2[:], hb[:], AF.Copy,
                                         scale=bsc[:, t : t + 1])
                    nc.vector.tensor_tensor(h2_all[:, t, :], t1[:], t2[:], ALU.add)
                    square_accum(h2_all[:, t, :], hn2[:, t : t + 1], L + f"h2{t}")

                # ---- SB3: proj + logmap0 scale ----
                hn = sc(L + "hn")
                nc.scalar.activation(hn[:], hn2[:], AF.Sqrt)
                rhn = sc(L + "rhn")
                clamp_recip(rhn, hn, L + "rhn")
                f2 = sc(L + "f2")
                nc.vector.tensor_scalar(f2[:], rhn[:], MAXN, 1.0, ALU.mult, ALU.min)
                m = sc(L + "m")
                nc.vector.tensor_scalar_min(m[:], hn[:], MAXN)
                rm = sc(L + "rm")
                clamp_recip(rm, m, L + "rm")
                atm = sc(L + "atm")
                artanh_ln(atm, m, L + "atm")
                g = sc(L + "g")
                nc.vector.tensor_tensor(g[:], atm[:], rm[:], ALU.mult)
                nc.vector.tensor_tensor(g[:], g[:], f2[:], ALU.mult)
                nc.vector.tensor_scalar_mul(g[:], g[:], 0.5)

                # ---- per tile: xt = g * h2 (tangent features) ----
                mmdt_ = _mm_bir_dtype()
                xt_all = persist.tile([128, NT, D], mmdt_, name=L + "xt", tag="bigC")
                for t in range(NT):
                    nc.vector.tensor_scalar_mul(xt_all[:, t, :], h2_all[:, t, :],
                                                g[:, t : t + 1])

                if DEBUG and li == 0:
                    nc.sync.dma_start(dbg["dbg_mx"].ap().rearrange("t p d -> p t d"),
                                      mx_all[:])
                    nc.sync.dma_start(dbg["dbg_xt"].ap().rearrange("t p d -> p t d"),
                                      xt_all[:])
                # ---- AllGather tangent features, chunked and overlapped ----
                # Split the gather into NCH chunks of TPC local tiles so the
                # spmm over chunk j runs while chunk j+1 is still gathering
                # (collectives run on TOPSP/SDMA, free of the 5 engines).
                NCH = 5
                TPC = NT // NCH
                xt_full = persist.tile([128, KT, D], mmdt_, name="xt_full",
                                       tag="xt_full")
                xtf_view = xt_full[:].rearrange("p (c t) d -> p c t d", t=NT)
                for j in range(NCH):
                    agin = dram.tile([TPC, 128, D], mmdt_, name=f"{L}agin{j}",
                                     tag=f"agin{j}")
                    agout = dram.tile([NCORES * TPC, 128, D], mmdt_,
                                      name=f"{L}agout{j}", tag=f"agout{j}",
                                      addr_space="Shared")
                    nc.sync.dma_start(agin[:].rearrange("t p d -> p t d"),
                                      xt_all[:, j * TPC : (j + 1) * TPC, :])
                    nc.gpsimd.collective_compute(
                        "AllGather", ALU.bypass,
                        replica_groups=[list(range(NCORES))],
                        ins=[agin[:].opt()], outs=[agout[:].opt()])
                    agout_v = agout[:].rearrange("(c t) p d -> p c t d", t=TPC)
                    for i in range(TPC):
                        nc.sync.dma_start(
                            xtf_view[:, :, j * TPC + i, :],
                            agout_v[:, :, i, :])
                if DEBUG and li == 0:
                    nc.sync.dma_start(dbg["dbg_xtf"].ap().rearrange("t p d -> p t d"),
                                      xt_full[:])

                # ---- spmm: agg[dst, f] = sum_src AT[src, dst] xt[src, f] ----
                # k-tiles grouped by AG chunk: chunk j supplies global k-tiles
                # {c*NT + j*TPC + i}. PSUM 'start' clears the whole 2KB bank;
                # tiles t, t+1 share a bank, so only the very first matmul of
                # each even tile issues start=True.
                pagg = psagg.tile([128, NT, D], F32, name="pagg", tag="pagg")
                k_iter = 0
                for j in range(NCH):
                    for c in range(NCORES):
                        for i in range(TPC):
                            kt = c * NT + j * TPC + i
                            at_k = atp.tile([128, PC], mmdt_, name="at_k",
                                            tag="at_k")
                            nc.sync.dma_start(at_k[:], a_d.ap()[kt])
                            for t in range(NT):
                                nc.tensor.matmul(
                                    pagg[:, t, :],
                                    at_k[:, t * 128 : (t + 1) * 128],
                                    xt_full[:, kt, :],
                                    start=(k_iter == 0 and t % 2 == 0),
                                    stop=(k_iter == KT - 1),
                                    skip_group_check=True)
                            k_iter += 1

                # ---- HypAct ----
                if DEBUG and li == 0:
                    agg_sb = persist.tile([128, NT, D], F32, name="agg_sb")
                    for t in range(NT):
                        nc.vector.tensor_copy(agg_sb[:, t, :], pagg[:, t, :])
                    nc.sync.dma_start(dbg["dbg_agg"].ap().rearrange("t p d -> p t d"),
                                      agg_sb[:])
                r2 = sc(L + "r2")
                for t in range(NT):
                    square_accum(pagg[:, t, :], r2[:, t : t + 1], L + f"agg{t}")
                rn = sc(L + "rn")
                nc.scalar.activation(rn[:], r2[:], AF.Sqrt)
                rrn = sc(L + "rrn")
                clamp_recip(rrn, rn, L + "rrn")
                th2 = sc(L + "th2")
                nc.scalar.activation(th2[:], rn[:], AF.Tanh)
                m1 = sc(L + "m1")
                nc.vector.tensor_scalar_min(m1[:], th2[:], MAXN)
                rm1 = sc(L + "rm1")
                clamp_recip(rm1, m1, L + "rm1")
                s1 = sc(L + "s1")
                nc.vector.tensor_tensor(s1[:], m1[:], rrn[:], ALU.mult)
                atq = sc(L + "atq")
                artanh_ln(atq, m1, L + "atq")
                qs = sc(L + "qs")
                nc.vector.tensor_tensor(qs[:], s1[:], atq[:], ALU.mult)
                nc.vector.tensor_tensor(qs[:], qs[:], rm1[:], ALU.mult)
                nc.vector.tensor_scalar_mul(qs[:], qs[:], 0.5)

                xt2_all = persist.tile([128, NT, D], F32, name=L + "xt2", tag="bigD")
                n2b = sc(L + "n2b")
                for t in range(NT):
                    # relu(qs*agg) on DVE (PSUM src), freeing the ACT engine
                    nc.vector.tensor_scalar(xt2_all[:, t, :], pagg[:, t, :],
                                            qs[:, t : t + 1], 0.0,
                                            ALU.mult, ALU.max)
                    square_accum(xt2_all[:, t, :], n2b[:, t : t + 1], L + f"xb{t}")

                un2 = sc(L + "un2")
                nc.scalar.activation(un2[:], n2b[:], AF.Sqrt)
                run2 = sc(L + "run2")
                clamp_recip(run2, un2, L + "run2")
                th3 = sc(L + "th3")
                nc.scalar.activation(th3[:], un2[:], AF.Tanh)
                mm2 = sc(L + "mm2")
                nc.vector.tensor_scalar_min(mm2[:], th3[:], MAXN)
                ss = sc(L + "ss")
                nc.vector.tensor_tensor(ss[:], mm2[:], run2[:], ALU.mult)

                e_all = persist.tile([128, NT, D], F32, name=L + "e", tag="bigE")
                for t in range(NT):
                    nc.vector.tensor_scalar_mul(e_all[:, t, :], xt2_all[:, t, :],
                                                ss[:, t : t + 1])
                nc.sync.dma_start(e_out_d.ap().rearrange("t p d -> p t d"),
                                  e_all[:])
                return e_all, mm2

            if DEBUG:
                nc.sync.dma_start(dbg["dbg_h"].ap().rearrange("t p d -> p t d"),
                                  h_all[:])
            e1_all, n1 = layer(0, h_all, mn0, e1_d)
            layer(1, e1_all, n1, e2_d)

    nc.compile()
    return nc


def _hyp_bias(b):
    """proj(expmap0(b, c=1), c=1) in float32, mirroring the reference."""
    b = b.astype(np.float32)
    un = np.maximum(np.sqrt((b * b).sum()), np.float32(MINN)).astype(np.float32)
    h = (np.tanh(un) * b / un).astype(np.float32)
    n = np.maximum(np.sqrt((h * h).sum()), np.float32(MINN)).astype(np.float32)
    if n > np.float32(MAXN):
        h = (h / n * np.float32(MAXN)).astype(np.float32)
    return h


def prepare_inputs(x, W1, b1, W2, b2, edge_index, edge_weight):
    mmnp = _mm_np_dtype()
    x = np.asarray(x, np.float32)
    W1 = np.asarray(W1, np.float32)
    W2 = np.asarray(W2, np.float32)
    b1 = np.asarray(b1, np.float32)
    b2 = np.asarray(b2, np.float32)
    ew = np.asarray(edge_weight, np.float32)
    src = np.asarray(edge_index[0], np.int64)
    dst = np.asarray(edge_index[1], np.int64)

    AT = np.zeros((NP, NP), np.float32)
    np.add.at(AT, (src, dst), ew)

    xfull = np.zeros((NP, D), np.float32)
    xfull[:N] = x

    hb1 = _hyp_bias(b1)
    hb2 = _hyp_bias(b2)
    y2s = (float((hb1.astype(np.float64) ** 2).sum()),
           float((hb2.astype(np.float64) ** 2).sum()))

    w1t = np.ascontiguousarray(W1.T).reshape(2, 128, D)
    w2t = np.ascontiguousarray(W2.T).reshape(2, 128, D)
    hb1_b = np.tile(hb1[None, :], (128, 1)).astype(np.float32)
    hb2_b = np.tile(hb2[None, :], (128, 1)).astype(np.float32)

    in_maps = []
    for c in range(NCORES):
        ac = np.ascontiguousarray(
            AT[:, c * PC : (c + 1) * PC]).reshape(KT, 128, PC).astype(mmnp)
        xcr = xfull[c * PC : (c + 1) * PC].reshape(NT, 128, D)
        in_maps.append({
            "xc": np.ascontiguousarray(xcr),
            "a": ac,
            "w1t": w1t, "w2t": w2t,
            "hb1": hb1_b, "hb2": hb2_b,
        })
    return in_maps, y2s


def assemble(results):
    e1 = np.concatenate([r["e1"].reshape(PC, D) for r in results], 0)[:N]
    e2 = np.concatenate([r["e2"].reshape(PC, D) for r in results], 0)[:N]
    return np.stack([e1, e2], 0).astype(np.float32)


def run(inputs, trace=False):
    in_maps, y2s = prepare_inputs(**inputs)
    nc = build_nc(y2s)
    res = run_bass_kernel_spmd(nc, in_maps, core_ids=list(range(NCORES)),
                               trace=trace)
    return assemble(res.results), res


def kernel(**inputs):
    out, _ = run(inputs, trace=False)
    return out



# revision 10
# speedup vs baseline: 1.0526x; 1.0526x over previous
"""HGCN (hyperbolic GCN) 2-layer forward for Trainium2, 8 NeuronCores.

Strategy (graph-parallel, dense-spmm), v2 — pipelined rewrite:
  - Nodes padded 10000 -> 10240, sharded 1280/core. segment_sum is a dense
    matmul agg = A @ xt; each core streams its [10240 x 1280] bf16 column
    slice of A^T from HBM, in a per-core slot order: the core's own 10
    k-tiles first (usable before any AllGather lands), then remote k-tiles
    grouped by AllGather chunk.
  - The AllGather of the [1280, 256] tangent features is chunked; chunk
    j's k-tiles are matmul'd while chunk j+1 is still on the fabric. The
    last chunk is processed dst-tile-major so HypAct for dst tile t starts
    right after its final matmul, hiding the elementwise phase under the
    spmm of later tiles and the next layer's AllGather.
  - All transcendentals come from the single natural_log_exp table set:
    sqrt(x) = exp(0.5 ln x), tanh(x) = 1 - 2/(e^{2x}+1), artanh via ln.
    artanh(min(tanh(u), MAXN)) folds to min(u, ATMAX), which collapses
    HypAgg's expmap0/proj/logmap0 chain into one scale qs = min(ATMAX/rn, 1).
    Zero activation-table switches after the first load.
  - Per-node scalar chains run on [128, 5] half-shards (2 groups of 5 node
    tiles) so group 0's chain overlaps group 1's matmuls.

kernel(**inputs) takes the FULL unsharded inputs and returns [2, N, D].
"""

import math
import sys

import numpy as np

for _p in ("/opt/trn_rl_repo",):
    if _p not in sys.path:
        sys.path.append(_p)

import concourse.bass as bass  # noqa: E402
import concourse.tile as tile  # noqa: E402
from concourse import bacc, mybir  # noqa: E402
from concourse.bass_utils import run_bass_kernel_spmd  # noqa: E402
from concourse.masks import make_identity  # noqa: E402

AF = mybir.ActivationFunctionType
ALU = mybir.AluOpType
F32 = mybir.dt.float32
BF16 = mybir.dt.bfloat16

NCORES = 8
N = 10000
D = 256
NP = 10240
PC = NP // NCORES      # 1280 nodes per core
NT = PC // 128         # 10 node tiles per core
KT = NP // 128         # 80 contraction tiles
MAXN = 1.0 - 4e-3      # PROJ_EPS clip for c=1
ATMAX = float(np.arctanh(np.float32(MAXN)))  # artanh of the proj clip
MINN = 1e-15
UCLAMP = 30.0          # tanh-arg clamp before exp(2x)
EPS2 = 1e-30           # norm^2 floor before ln

GROUPS = [(0, 5), (5, 5)]           # node-tile groups for scalar chains
CHUNKS = [(0, 5), (5, 4), (9, 1)]   # AllGather chunks (start, len)


def build_nc(y2s):
    """Build the per-core Bass program. y2s = (||hyp_b1||^2, ||hyp_b2||^2)."""
    nc = bacc.Bacc("TRN2", target_bir_lowering=False, debug=False,
                   num_devices=NCORES)

    xc = nc.dram_tensor("xc", [NT, 128, D], F32, kind="ExternalInput")
    a_d = nc.dram_tensor("a", [KT, 128, PC], BF16, kind="ExternalInput")
    w1t = nc.dram_tensor("w1t", [2, 128, D], F32, kind="ExternalInput")
    w2t = nc.dram_tensor("w2t", [2, 128, D], F32, kind="ExternalInput")
    hb1 = nc.dram_tensor("hb1", [128, D], F32, kind="ExternalInput")
    hb2 = nc.dram_tensor("hb2", [128, D], F32, kind="ExternalInput")
    e1_d = nc.dram_tensor("e1", [NT, 128, D], F32, kind="ExternalOutput")
    e2_d = nc.dram_tensor("e2", [NT, 128, D], F32, kind="ExternalOutput")

    with tile.TileContext(nc) as tc:
        with (
            tc.tile_pool(name="const", bufs=1) as const,
            tc.tile_pool(name="persist", bufs=1) as persist,
            tc.tile_pool(name="sqp", bufs=4) as sqp,
            tc.tile_pool(name="grp", bufs=2) as grp,
            tc.tile_pool(name="htp", bufs=4) as htp,
            tc.tile_pool(name="atp", bufs=20) as atp,
            tc.tile_pool(name="pst", bufs=2, space="PSUM") as pst,
            tc.tile_pool(name="psmx", bufs=1, space="PSUM") as psmx,
            tc.tile_pool(name="psagg", bufs=1, space="PSUM") as psagg,
            tc.tile_pool(name="dram", bufs=1, space="DRAM") as dram,
        ):
            ident = const.tile([128, 128], F32, name="ident")
            make_identity(nc, ident)

            w_sb = []
            for li, wd in enumerate((w1t, w2t)):
                w = const.tile([128, 2, D], F32, name=f"w{li}")
                nc.sync.dma_start(w[:], wd.ap().rearrange("k p n -> p k n"))
                w_sb.append(w)
            hb_sb = []
            for li, hd in enumerate((hb1, hb2)):
                h = const.tile([128, D], F32, name=f"hb{li}")
                nc.sync.dma_start(h[:], hd.ap())
                hb_sb.append(h)

            def sc(name):
                return persist.tile([128, NT], F32, name=name)

            def scratch():
                return sqp.tile([128, D], F32, name="scr", tag="scr")

            def norm2_dve(src_ap, accum_ap):
                """accum = sum(src^2) on the ACT engine (baseline-proven)."""
                s = scratch()
                nc.scalar.activation(s[:], src_ap, AF.Square,
                                     accum_out=accum_ap)

            def norm2_act(src_ap, accum_ap):
                """accum = sum(src^2) on the ACT engine."""
                s = scratch()
                nc.scalar.activation(s[:], src_ap, AF.Square,
                                     accum_out=accum_ap)

            def dot_hb(src_ap, hb, accum_ap):
                s = scratch()
                nc.vector.tensor_tensor(s[:], src_ap, hb[:], ALU.mult)
                nc.vector.reduce_sum(accum_ap, s[:],
                                     axis=mybir.AxisListType.X)

            def rsqrt_sl(dst_sl, src_sl, tmp):
                """dst = 1/sqrt(max(src, EPS2)) via ln+exp (no table switch).
                tmp: same-shape scratch slice for the clamped input."""
                nc.vector.tensor_scalar_max(tmp, src_sl, EPS2)
                nc.scalar.activation(dst_sl, tmp, AF.Ln)
                nc.scalar.activation(dst_sl, dst_sl, AF.Exp, scale=-0.5)

            def tanh_sl(dst_sl, src_sl, tmp):
                """dst = tanh(src) for src >= 0, via exp (no table switch)."""
                nc.vector.tensor_scalar_min(tmp, src_sl, UCLAMP)
                nc.scalar.activation(dst_sl, tmp, AF.Exp, scale=2.0)
                nc.vector.tensor_scalar_add(dst_sl, dst_sl, 1.0)
                nc.vector.reciprocal(dst_sl, dst_sl)
                nc.vector.tensor_scalar(dst_sl, dst_sl, -2.0, 1.0,
                                        ALU.mult, ALU.add)

            # ---------------- encode: h = proj(expmap0(x)) ----------------
            x_sb = persist.tile([128, NT, D], F32, name="x_sb", tag="bigA")
            nc.sync.dma_start(x_sb[:], xc.ap().rearrange("t p d -> p t d"))
            h_all = persist.tile([128, NT, D], F32, name="h_all", tag="bigB")
            xn2 = sc("xn2")
            for t in range(NT):
                norm2_dve(x_sb[:, t, :], xn2[:, t:t + 1])
            rux = sc("rux")      # 1/||x||
            tmp0 = sc("tmp0")
            rsqrt_sl(rux[:], xn2[:], tmp0[:])
            un = sc("un")        # ||x||
            nc.vector.tensor_tensor(un[:], xn2[:], rux[:], ALU.mult)
            thx = sc("thx")
            tanh_sl(thx[:], un[:], tmp0[:])
            hn = sc("hn0")
            nc.vector.tensor_scalar_min(hn[:], thx[:], MAXN)
            hat = sc("hat0")
            nc.vector.tensor_scalar_min(hat[:], un[:], ATMAX)
            s0 = sc("s0")
            nc.vector.tensor_tensor(s0[:], hn[:], rux[:], ALU.mult)
            for t in range(NT):
                nc.vector.tensor_scalar_mul(h_all[:, t, :], x_sb[:, t, :],
                                            s0[:, t:t + 1])

            def layer(li, h_in, hn, hat, e_out_d, h_out_tag):
                """One HGCN layer. h_in [128,NT,D] on-ball, hn its row norms,
                hat = artanh(hn). Returns (e_all, hn_next, hat_next)."""
                L = f"l{li}_"
                w = w_sb[li]
                hb = hb_sb[li]
                y2 = float(y2s[li])

                mx_all = persist.tile([128, NT, D], F32, name=L + "mx",
                                      tag="bigA")
                xt_all = persist.tile([128, NT, D], BF16, name=L + "xt",
                                      tag="xtA")
                mn2 = sc(L + "mn2")
                ryp = sc(L + "ryp")
                hn2 = sc(L + "hn2")
                rsq = sc(L + "rsq")
                u_ = sc(L + "u")
                th = sc(L + "th")
                sres = sc(L + "sres")
                f1 = sc(L + "f1")
                x2 = sc(L + "x2")
                xy = sc(L + "xy")
                asc = sc(L + "asc")
                bsc = sc(L + "bsc")
                rsq2 = sc(L + "rsq2")
                m_ = sc(L + "m")
                gg = sc(L + "gg")
                tmp = sc(L + "tmp")
                tmp2 = sc(L + "tmp2")
                hq = sc(L + "hq")

                h2gs = {}

                def pre_group(gi, g0, gn):
                    B = slice(g0, g0 + gn)
                    # hquot = hat / max(hn, MINN), per group slice
                    nc.vector.tensor_scalar_max(tmp[:, B], hn[:, B], MINN)
                    nc.vector.reciprocal(tmp[:, B], tmp[:, B])
                    nc.vector.tensor_tensor(hq[:, B], hat[:, B], tmp[:, B],
                                            ALU.mult)
                    for t in range(g0, g0 + gn):
                        hT = htp.tile([128, 2, 128], F32, name="hT", tag="hT")
                        for kc in range(2):
                            psT = pst.tile([128, 128], F32, name="psT",
                                           tag="psT")
                            nc.tensor.transpose(
                                psT[:], h_in[:, t, kc * 128:(kc + 1) * 128],
                                ident[:])
                            nc.vector.tensor_copy(hT[:, kc, :], psT[:])
                        pmx = psmx.tile([128, D], F32, name="pmx", tag="pmx")
                        nc.tensor.matmul(pmx[:], hT[:, 0, :], w[:, 0, :],
                                         start=True, stop=False)
                        nc.tensor.matmul(pmx[:], hT[:, 1, :], w[:, 1, :],
                                         start=False, stop=True)
                        nc.vector.tensor_copy(mx_all[:, t, :], pmx[:])
                        norm2_act(pmx[:], mn2[:, t:t + 1])
                    # SB1: mobius_matvec scalars
                    rsqrt_sl(rsq[:, B], mn2[:, B], tmp[:, B])
                    nc.vector.tensor_tensor(u_[:, B], mn2[:, B], rsq[:, B],
                                            ALU.mult)   # sqrt(mn2)
                    nc.vector.tensor_tensor(u_[:, B], u_[:, B], hq[:, B],
                                            ALU.mult)
                    tanh_sl(th[:, B], u_[:, B], tmp[:, B])
                    nc.vector.tensor_tensor(sres[:, B], th[:, B], rsq[:, B],
                                            ALU.mult)
                    nc.vector.tensor_scalar_max(tmp[:, B], th[:, B], MINN)
                    nc.vector.reciprocal(tmp[:, B], tmp[:, B])
                    nc.vector.tensor_scalar(f1[:, B], tmp[:, B], MAXN, 1.0,
                                            ALU.mult, ALU.min)
                    nc.vector.tensor_scalar_min(tmp[:, B], th[:, B], MAXN)
                    nc.vector.tensor_tensor(x2[:, B], tmp[:, B], tmp[:, B],
                                            ALU.mult)
                    for t in range(g0, g0 + gn):
                        dot_hb(mx_all[:, t, :], hb, ryp[:, t:t + 1])
                    # SB2: mobius_add coefficients
                    nc.vector.tensor_tensor(xy[:, B], ryp[:, B], sres[:, B],
                                            ALU.mult)
                    nc.vector.tensor_tensor(xy[:, B], xy[:, B], f1[:, B],
                                            ALU.mult)
                    # alpha = (1 + 2 xy + y2) * f1 ; den = 1 + 2 xy + x2 y2
                    nc.vector.tensor_scalar(tmp[:, B], xy[:, B], 2.0,
                                            1.0 + y2, ALU.mult, ALU.add)
                    nc.vector.tensor_tensor(tmp[:, B], tmp[:, B], f1[:, B],
                                            ALU.mult)   # alpha
                    nc.vector.tensor_scalar(tmp2[:, B], x2[:, B], y2, 1.0,
                                            ALU.mult, ALU.add)
                    nc.vector.tensor_scalar(bsc[:, B], xy[:, B], 2.0, 0.0,
                                            ALU.mult, ALU.add)  # 2 xy
                    nc.vector.tensor_tensor(tmp2[:, B], tmp2[:, B],
                                            bsc[:, B], ALU.add)  # den
                    nc.vector.tensor_scalar_max(tmp2[:, B], tmp2[:, B], MINN)
                    nc.vector.reciprocal(tmp2[:, B], tmp2[:, B])  # dinv
                    nc.vector.tensor_tensor(asc[:, B], tmp[:, B], tmp2[:, B],
                                            ALU.mult)
                    nc.vector.tensor_tensor(asc[:, B], asc[:, B], sres[:, B],
                                            ALU.mult)
                    nc.vector.tensor_scalar(tmp[:, B], x2[:, B], -1.0, 1.0,
                                            ALU.mult, ALU.add)  # beta
                    nc.vector.tensor_tensor(bsc[:, B], tmp[:, B], tmp2[:, B],
                                            ALU.mult)
                    # h2 = asc*mx + bsc*hb ; |h2|^2
                    h2g = grp.tile([128, gn, D], F32, name=L + f"h2g{gi}",
                                   tag=f"h2g{gi % 2}")
                    h2gs[gi] = h2g
                    for t in range(g0, g0 + gn):
                        tl = t - g0
                        t2 = sqp.tile([128, D], F32, name="t2t", tag="t2t")
                        nc.scalar.activation(t2[:], hb[:], AF.Copy,
                                             scale=bsc[:, t:t + 1])
                        nc.vector.tensor_scalar_mul(h2g[:, tl, :],
                                                    mx_all[:, t, :],
                                                    asc[:, t:t + 1])
                        nc.vector.tensor_tensor(h2g[:, tl, :], h2g[:, tl, :],
                                                t2[:], ALU.add)
                        norm2_act(h2g[:, tl, :], hn2[:, t:t + 1])
                    # SB3: proj + logmap0 scale
                    rsqrt_sl(rsq2[:, B], hn2[:, B], tmp[:, B])
                    nc.vector.tensor_tensor(m_[:, B], hn2[:, B], rsq2[:, B],
                                            ALU.mult)   # |h2|
                    nc.vector.tensor_scalar_min(m_[:, B], m_[:, B], MAXN)
                    # artanh(m) = 0.5 (ln(1+m) - ln(1-m))
                    nc.scalar.activation(tmp[:, B], m_[:, B], AF.Ln,
                                         bias=1.0, scale=-1.0)  # ln(1-m)
                    nc.scalar.activation(tmp2[:, B], m_[:, B], AF.Ln,
                                         bias=1.0, scale=1.0)   # ln(1+m)
                    nc.vector.tensor_tensor(tmp2[:, B], tmp2[:, B],
                                            tmp[:, B], ALU.subtract)
                    nc.vector.tensor_scalar_max(tmp[:, B], m_[:, B], MINN)
                    nc.vector.reciprocal(tmp[:, B], tmp[:, B])
                    nc.vector.tensor_tensor(gg[:, B], tmp2[:, B], tmp[:, B],
                                            ALU.mult)
                    # * f2 = min(MAXN * rsq2, 1), * 0.5
                    nc.vector.tensor_scalar(tmp[:, B], rsq2[:, B], MAXN, 1.0,
                                            ALU.mult, ALU.min)
                    nc.vector.tensor_tensor(gg[:, B], gg[:, B], tmp[:, B],
                                            ALU.mult)
                    nc.vector.tensor_scalar_mul(gg[:, B], gg[:, B], 0.5)
                    for t in range(g0, g0 + gn):
                        tl = t - g0
                        nc.vector.tensor_scalar_mul(xt_all[:, t, :],
                                                    h2g[:, tl, :],
                                                    gg[:, t:t + 1])

                # ---- PRE groups + AllGather chunks as tiles complete ----
                done = 0
                chunk_q = list(CHUNKS)
                ag_bufs = []
                for gi, (g0, gn) in enumerate(GROUPS):
                    pre_group(gi, g0, gn)
                    done = g0 + gn
                    while chunk_q and chunk_q[0][0] + chunk_q[0][1] <= done:
                        a0, an = chunk_q.pop(0)
                        agin = dram.tile([an, 128, D], BF16,
                                         name=f"{L}agi{a0}", tag=f"agi{a0}")
                        agout = dram.tile([NCORES * an, 128, D], BF16,
                                          name=f"{L}ago{a0}", tag=f"ago{a0}",
                                          addr_space="Shared")
                        nc.sync.dma_start(
                            agin[:].rearrange("t p d -> p t d"),
                            xt_all[:, a0:a0 + an, :])
                        nc.gpsimd.collective_compute(
                            "AllGather", ALU.bypass,
                            replica_groups=[list(range(NCORES))],
                            ins=[agin[:].opt()], outs=[agout[:].opt()])
                        ag_bufs.append((a0, an, agout))

                # ---- gather readback + spmm ----
                xt_full = persist.tile([128, NCORES, NT, D], BF16,
                                       name="xt_full", tag="xtf")
                pagg = psagg.tile([128, NT, D], F32, name="pagg", tag="pagg")

                def mm(s, rhs_ap, t, first_s, last_s):
                    nc.tensor.matmul(
                        pagg[:, t, :],
                        at_sb[s][:, t * 128:(t + 1) * 128],
                        rhs_ap,
                        start=(first_s and t % 2 == 0),
                        stop=last_s,
                        skip_group_check=True)

                at_sb = {}

                def load_at(s):
                    at = atp.tile([128, PC], BF16, name="at_k", tag="at_k")
                    nc.sync.dma_start(at[:], a_d.ap()[s])
                    at_sb[s] = at

                s_next = 0
                # chunk-major slots, k-major except the last (dst-major tail)
                for ci, (a0, an, agout) in enumerate(ag_bufs):
                    agv = agout[:].rearrange("(c t) p d -> p c t d", t=an)
                    for i in range(an):
                        nc.sync.dma_start(
                            xt_full[:, :, a0 + i, :], agv[:, :, i, :])
                    slots = []
                    for c in range(NCORES):
                        for i in range(an):
                            slots.append((s_next, c, a0 + i))
                            s_next += 1
                    for s, c, i in slots:
                        load_at(s)
                    last_chunk = ci == len(ag_bufs) - 1
                    if not last_chunk:
                        for s, c, i in slots:
                            for t in range(NT):
                                mm(s, xt_full[:, c, i, :], t, s == 0, False)
                    else:
                        for t in range(NT):
                            for si, (s, c, i) in enumerate(slots):
                                mm(s, xt_full[:, c, i, :], t, s == 0,
                                   si == len(slots) - 1)

                # ---- POST: HypAct per group, pipelined off the spmm tail --
                e_all = persist.tile([128, NT, D], F32, name=L + "e",
                                     tag=h_out_tag)
                r2 = sc(L + "r2")
                n2b = sc(L + "n2b")
                qs = sc(L + "qs")
                rs2 = sc(L + "rs2")
                un2 = sc(L + "un2")
                th3 = sc(L + "th3")
                hnn = sc(L + "hnn")
                hatn = sc(L + "hatn")
                ss = sc(L + "ss")
                for gi, (g0, gn) in enumerate(GROUPS):
                    B = slice(g0, g0 + gn)
                    for t in range(g0, g0 + gn):
                        norm2_act(pagg[:, t, :], r2[:, t:t + 1])
                    rsqrt_sl(qs[:, B], r2[:, B], tmp[:, B])  # 1/rn
                    nc.vector.tensor_scalar(qs[:, B], qs[:, B], ATMAX, 1.0,
                                            ALU.mult, ALU.min)
                    xt2g = grp.tile([128, gn, D], F32, name=L + f"x2g{gi}",
                                    tag=f"x2g{gi % 2}")
                    for t in range(g0, g0 + gn):
                        tl = t - g0
                        nc.vector.tensor_scalar(xt2g[:, tl, :], pagg[:, t, :],
                                                qs[:, t:t + 1], 0.0,
                                                ALU.mult, ALU.max)
                        norm2_act(xt2g[:, tl, :], n2b[:, t:t + 1])
                    rsqrt_sl(rs2[:, B], n2b[:, B], tmp[:, B])
                    nc.vector.tensor_tensor(un2[:, B], n2b[:, B], rs2[:, B],
                                            ALU.mult)
                    tanh_sl(th3[:, B], un2[:, B], tmp[:, B])
                    nc.vector.tensor_scalar_min(hnn[:, B], th3[:, B], MAXN)
                    nc.vector.tensor_scalar_min(hatn[:, B], un2[:, B], ATMAX)
                    nc.vector.tensor_tensor(ss[:, B], hnn[:, B], rs2[:, B],
                                            ALU.mult)
                    for t in range(g0, g0 + gn):
                        tl = t - g0
                        nc.vector.tensor_scalar_mul(e_all[:, t, :],
                                                    xt2g[:, tl, :],
                                                    ss[:, t:t + 1])
                    nc.sync.dma_start(
                        e_out_d.ap().rearrange("t p d -> p t d")[:, B, :],
                        e_all[:, B, :])
                return e_all, hnn, hatn

            e1_all, n1, a1 = layer(0, h_all, hn, hat, e1_d, "bigC")
            layer(1, e1_all, n1, a1, e2_d, "bigB")

    nc.compile()
    return nc


def _hyp_bias(b):
    """proj(expmap0(b, c=1), c=1) in float32, mirroring the reference."""
    b = b.astype(np.float32)
    un = np.maximum(np.sqrt((b * b).sum()), np.float32(MINN)).astype(np.float32)
    h = (np.tanh(un) * b / un).astype(np.float32)
    n = np.maximum(np.sqrt((h * h).sum()), np.float32(MINN)).astype(np.float32)
    if n > np.float32(MAXN):
        h = (h / n * np.float32(MAXN)).astype(np.float32)
    return h


def prepare_inputs(x, W1, b1, W2, b2, edge_index, edge_weight):
    import ml_dtypes

    mmnp = np.dtype(ml_dtypes.bfloat16)
    x = np.asarray(x, np.float32)
    W1 = np.asarray(W1, np.float32)
    W2 = np.asarray(W2, np.float32)
    b1 = np.asarray(b1, np.float32)
    b2 = np.asarray(b2, np.float32)
    ew = np.asarray(edge_weight, np.float32)
    src = np.asarray(edge_index[0], np.int64)
    dst = np.asarray(edge_index[1], np.int64)

    AT = np.zeros((NP, NP), np.float32)
    np.add.at(AT, (src, dst), ew)

    xfull = np.zeros((NP, D), np.float32)
    xfull[:N] = x

    hb1 = _hyp_bias(b1)
    hb2 = _hyp_bias(b2)
    y2s = (float((hb1.astype(np.float64) ** 2).sum()),
           float((hb2.astype(np.float64) ** 2).sum()))

    w1t = np.ascontiguousarray(W1.T).reshape(2, 128, D)
    w2t = np.ascontiguousarray(W2.T).reshape(2, 128, D)
    hb1_b = np.tile(hb1[None, :], (128, 1)).astype(np.float32)
    hb2_b = np.tile(hb2[None, :], (128, 1)).astype(np.float32)

    in_maps = []
    for c in range(NCORES):
        acol = AT[:, c * PC:(c + 1) * PC]  # [NP src, PC dst]
        # slot order: AG-chunk-major, matching the device's consumption
        slots = [cc * NT + i
                 for a0, an in CHUNKS
                 for cc in range(NCORES)
                 for i in range(a0, a0 + an)]
        assert len(slots) == KT
        ac = np.stack([acol[g * 128:(g + 1) * 128, :] for g in slots], 0)
        xcr = xfull[c * PC:(c + 1) * PC].reshape(NT, 128, D)
        in_maps.append({
            "xc": np.ascontiguousarray(xcr),
            "a": np.ascontiguousarray(ac).astype(mmnp),
            "w1t": w1t, "w2t": w2t,
            "hb1": hb1_b, "hb2": hb2_b,
        })
    return in_maps, y2s


def assemble(results):
    e1 = np.concatenate([r["e1"].reshape(PC, D) for r in results], 0)[:N]
    e2 = np.concatenate([r["e2"].reshape(PC, D) for r in results], 0)[:N]
    return np.stack([e1, e2], 0).astype(np.float32)


def run(inputs, trace=False):
    in_maps, y2s = prepare_inputs(**inputs)
    nc = build_nc(y2s)
    res = run_bass_kernel_spmd(nc, in_maps, core_ids=list(range(NCORES)),
                               trace=trace)
    return assemble(res.results), res


def kernel(**inputs):
    out, _ = run(inputs, trace=False)
    return out
